# revision 28
# baseline (speedup 1.0000x reference)
"""Trainium2 Bass kernel for nn_Capsule: capsule routing head.

Math: the einsum 'nco,pbo->bno' factorizes as xp[b,n,o] = W[n,o] * X[b,o]
with W = caps_weights.sum(c) (64x128) and X = x.sum(p) (256x128), so the
kernel is a memory-bound reduction of x (151 MB) followed by a tiny
per-batch routing loop.

Sharding: data-parallel over batch (dim 1 of x), 32 batch elements per
core; caps_weights replicated; no cross-core communication.

Per-core pipeline (v2):
  - x is streamed as 9 slabs of 120 partition-rows plus one 72-row
    remainder slab.  A DMA is split across SDMA engines by the largest
    divisor <= 16 of its top AP dim: 120 -> 15 engines (128 would put
    1/16 on engine 15, measured ~15% slower than engines 0-14; 124
    would split only 4 ways).  The 72-row remainder splits 12 ways.
  - Slabs 0-5 stream full-width (16KB/partition descriptors, line
    rate); the tail block {s6, s7, s8, rem} is split into 4 batch
    ranges of 8 (4KB descriptors -- 2KB ones measured only ~66% of
    line rate), so only the last range's reduction matmuls are exposed
    at stream end.  cst/caps_weights/remainder chunks ride the gpsimd
    (SWDGE) queue so the HWDGE rings carry nothing but x slabs.
  - Reduction via fp32r matmuls with one-hot-column stationary matrices
    into a single psum bank (32, 512), in three phases matched to
    stream arrival (slabs 0-2, 3-5, then 6-9 per range).
  - Routing runs in o-on-partitions layout: partition reductions and
    per-batch broadcasts are tiny matmuls (ones stationaries), softmax
    normalization is deferred into per-batch row scalars, and no
    mid-routing transposes are needed.  sqrt(q) = Exp(0.5*Ln(q)) keeps
    every ACT op in one table ('natural_log_exp_and_others').
"""

import numpy as np

# ---- problem constants (hardcoded per contract) ----
P_TOT = 1152
BATCH = 256
O = 128
N_CAPS = 64
CAPS_DIM = 16
ITERATIONS = 3
N_CORES = 8
B_LOC = BATCH // N_CORES          # 32 batch elements per core

PR = 120                          # partition rows per main slab
NS = 9                            # main slabs
REM = P_TOT - PR * NS             # 72-row remainder slab
NSL = 10                          # slab positions incl. zero-padded remainder
NFULL = 6                         # slabs streamed full-width (16KB descriptors)
NR = 4                            # batch ranges for the tail block {s6..s9}
BB = B_LOC // NR                  # batch elements per range (8)
SW = B_LOC * O                    # free width per slab position (4096)

_cache = {}


def _pin_act_table():
    """Force every ACT function onto the one table containing
    Exp+Ln+Square+Copy, so the kernel needs a single ACT_TABLE_LOAD."""
    import functools
    import concourse.hw_specs as hw_specs
    import concourse.bacc as bacc_mod

    if getattr(hw_specs.get_activation_tables, "_capsule_pinned", False):
        return
    orig = hw_specs.get_activation_tables

    @functools.cache
    def pinned(module_arch):
        tabs = orig(module_arch)
        keep = None
        for name, fns in tabs.items():
            names = {f.name for f in fns}
            if {"Exp", "Ln", "Square", "Copy", "Identity"} <= names:
                keep = name
                break
        if keep is None:
            return tabs
        return {n: (fns if n == keep else type(fns)()) for n, fns in tabs.items()}

    pinned._capsule_pinned = True
    hw_specs.get_activation_tables = pinned
    bacc_mod.get_activation_tables = pinned


def _build(debug=False):
    _pin_act_table()
    import concourse.bacc as bacc
    import concourse.tile as tile
    import concourse.mybir as mybir
    from concourse.masks import make_identity

    f32 = mybir.dt.float32
    f32r = mybir.dt.float32r
    AX = mybir.AxisListType
    AF = mybir.ActivationFunctionType
    OP = mybir.AluOpType

    nc = bacc.Bacc(None, target_bir_lowering=False)

    # x declared f32r: same bytes as fp32, feeds the fast fp32r matmul
    # path with no cast.
    x_in = nc.dram_tensor("x", [P_TOT, B_LOC, O], f32r, kind="ExternalInput")
    w_in = nc.dram_tensor("caps_weights", [N_CAPS, CAPS_DIM, O], f32,
                          kind="ExternalInput")
    # one-hot stationary source: (128, 63) with ones in column 31, so
    # cst[:p, 31-b : 63-b] is the one-hot-column-b matrix on p rows.
    cst_in = nc.dram_tensor("cst", [128, 2 * B_LOC - 1], f32r,
                            kind="ExternalInput")
    out_d = nc.dram_tensor("out", [B_LOC, O], f32, kind="ExternalOutput")
    if debug:
        dbg_x32 = nc.dram_tensor("dbg_x32", [B_LOC, O], f32,
                                 kind="ExternalOutput")
        dbg_lg = nc.dram_tensor("dbg_lg", [N_CAPS, B_LOC], f32,
                                kind="ExternalOutput")
        dbg_ex1 = nc.dram_tensor("dbg_ex1", [N_CAPS, B_LOC], f32,
                                 kind="ExternalOutput")
        dbg_r = {}
        for nm in ("r1", "r2", "cf1", "cf2"):
            dbg_r[nm] = nc.dram_tensor(f"dbg_{nm}", [1, B_LOC], f32,
                                       kind="ExternalOutput")
        dbg_lg2 = nc.dram_tensor("dbg_lg2", [N_CAPS, B_LOC], f32,
                                 kind="ExternalOutput")

    with tile.TileContext(nc) as tc:
        with (
            tc.tile_pool(name="xin", bufs=1) as xpool,
            tc.tile_pool(name="wrk", bufs=1) as wrk,
            tc.tile_pool(name="small", bufs=1) as small,
            tc.tile_pool(name="ps", bufs=1, space="PSUM") as ps,
        ):
            # ---- constants / weights ride the SWDGE (gpsimd) queue so
            # the two HWDGE rings carry nothing but x chunks.
            zpat = small.tile([128, 2 * B_LOC - 1], f32r)
            nc.gpsimd.dma_start(zpat[:], cst_in[:])
            w_sb = wrk.tile([N_CAPS, CAPS_DIM * O], f32)
            nc.gpsimd.dma_start(w_sb[:], w_in.rearrange("n c o -> n (c o)"))

            # ---- x stream.  Slab layout is b-major: position s holds
            # (120, 32b, 128o) at cols [s*4096, (s+1)*4096).  Slabs 0-5
            # stream full-width (16KB/partition descriptors, line rate);
            # the tail block {s6, s7, s8, rem} is split into 4 batch
            # ranges of 8 (4KB descriptors) so only the last range's
            # reduction matmuls are exposed at stream end.  The 72-row
            # remainder lives zero-padded at position 9; its chunks ride
            # the SWDGE queue.
            xa = xpool.tile([PR, NSL * SW], f32r, name="xa")
            xa_sl = xa[:].rearrange("p (s b o) -> p s b o", s=NSL, b=B_LOC)
            engs = [nc.sync, nc.scalar]
            for s in range(NFULL):
                engs[s % 2].dma_start(
                    xa_sl[:, s, :, :],
                    x_in[s * PR:(s + 1) * PR, :, :])
            for r in range(NR):
                for s in range(NFULL, NS):
                    engs[(s + r) % 2].dma_start(
                        xa_sl[:, s, r * BB:(r + 1) * BB, :],
                        x_in[s * PR:(s + 1) * PR, r * BB:(r + 1) * BB, :])
                nc.gpsimd.dma_start(
                    xa_sl[0:REM, NS, r * BB:(r + 1) * BB, :],
                    x_in[NS * PR:, r * BB:(r + 1) * BB, :])

            # ---- small constants ----
            ones_c = small.tile([128, 1], f32)      # column of ones
            nc.vector.memset(ones_c[:], 1.0)
            ones_r = small.tile([1, 128], f32)      # row of ones
            nc.vector.memset(ones_r[:], 1.0)
            nbias = small.tile([N_CAPS, 1], f32)    # softmax shift (see Exp)
            nc.vector.memset(nbias[:], -21.0)
            ident = small.tile([128, 128], f32)
            make_identity(nc, ident[:])

            # ---- capsule weight prep (overlaps the x stream) ----
            t1 = wrk.tile([N_CAPS, 8 * O], f32)
            nc.vector.tensor_tensor(t1[:], w_sb[:, :8 * O], w_sb[:, 8 * O:], OP.add)
            t2 = wrk.tile([N_CAPS, 4 * O], f32)
            nc.vector.tensor_tensor(t2[:], t1[:, :4 * O], t1[:, 4 * O:], OP.add)
            t3 = wrk.tile([N_CAPS, 2 * O], f32)
            nc.vector.tensor_tensor(t3[:], t2[:, :2 * O], t2[:, 2 * O:], OP.add)
            w_no = wrk.tile([N_CAPS, O], f32)          # W[n,o]
            nc.vector.tensor_tensor(w_no[:], t3[:, :O], t3[:, O:], OP.add)

            ps_wt = ps.tile([O, N_CAPS], f32, tag="ps_wt")
            nc.tensor.transpose(ps_wt[:], w_no[:], ident[:N_CAPS, :N_CAPS])
            wt_on = wrk.tile([O, N_CAPS], f32)          # W^T[o,n]
            nc.vector.tensor_copy(wt_on[:], ps_wt[:])
            # wbar[o] = mean_n W[n,o]  (column of per-o means)
            ps_wb = ps.tile([O, 1], f32, tag="ps_wb")
            nc.tensor.matmul(ps_wb[:], w_no[:], ones_c[:N_CAPS, :],
                             start=True, stop=True)
            wbar = wrk.tile([O, 1], f32)
            nc.scalar.activation(wbar[:], ps_wb[:], AF.Copy, scale=1.0 / N_CAPS)

            # ---- reduction: X[b,o] = sum_p x[p,b,o] ----
            # one-hot-column stationaries land row b of psum.  Three
            # phases matched to stream arrival: P1 = slabs 0-2 (384-wide,
            # after slab 2 lands), P2 = slabs 3-5, P3 per range = slabs
            # 6-8 plus the 72-row remainder (128-wide into sub-column 0).
            # psum sub-column j*128+o accumulates slabs with s = j mod 3;
            # all 128 matmuls accumulate into ONE bank.
            ps_x = ps.tile([B_LOC, 3 * O], f32, tag="ps_x")
            first = True
            for b in range(B_LOC):
                st = zpat[0:PR, B_LOC - 1 - b: 2 * B_LOC - 1 - b]
                nc.tensor.matmul(ps_x[:], st, xa_sl[:, 0:3, b, :],
                                 start=first, stop=False,
                                 skip_group_check=True)
                first = False
            for b in range(B_LOC):
                st = zpat[0:PR, B_LOC - 1 - b: 2 * B_LOC - 1 - b]
                nc.tensor.matmul(ps_x[:], st, xa_sl[:, 3:6, b, :],
                                 start=False, stop=False,
                                 skip_group_check=True)
            for r in range(NR):
                for bb in range(BB):
                    b = r * BB + bb
                    st = zpat[0:PR, B_LOC - 1 - b: 2 * B_LOC - 1 - b]
                    stR = zpat[0:REM, B_LOC - 1 - b: 2 * B_LOC - 1 - b]
                    last = (r == NR - 1 and bb == BB - 1)
                    nc.tensor.matmul(ps_x[:], st, xa_sl[:, 6:9, b, :],
                                     start=False, stop=False,
                                     skip_group_check=True)
                    nc.tensor.matmul(ps_x[:, 0:O], stR,
                                     xa_sl[0:REM, 9, b, :],
                                     start=False, stop=last,
                                     skip_group_check=True)

            # ---- X prep: X (b-part) -> XT, X2T (o-part) ----
            x32 = wrk.tile([B_LOC, O], f32)
            nc.vector.tensor_reduce(
                x32[:], ps_x[:].rearrange("p (j o) -> p o j", j=3),
                AX.X, OP.add)
            if debug:
                nc.sync.dma_start(dbg_x32[:], x32[:])
            ps_a = ps.tile([O, B_LOC], f32, tag="ps_a")
            nc.tensor.transpose(ps_a[:], x32[:], ident[:B_LOC, :B_LOC])
            xt = wrk.tile([O, B_LOC], f32)              # X^T[o,b]
            nc.vector.tensor_copy(xt[:], ps_a[:])
            x2t = wrk.tile([O, B_LOC], f32)             # (X^T)^2
            nc.scalar.activation(x2t[:], xt[:], AF.Square)

            # ---- routing, o-on-partitions ----
            # row scalars live on (1, B_LOC); partition reductions and
            # broadcasts are matmuls with ones stationaries.
            ROW = [1, B_LOC]

            def rowchain(qp, rprev):
                """scale-row coef = sqrt(q)/(1+q) (x rprev^2 deferred
                softmax normalization; rprev=None for iteration 0).
                qp: (1,32) psum with raw |u|^2 (missing rprev^2)."""
                lnq = wrk.tile(ROW, f32, tag="lnq")
                nc.scalar.activation(lnq[:], qp[:], AF.Ln)
                nrm = wrk.tile(ROW, f32, tag="nrm")     # sqrt(q_raw)
                nc.scalar.activation(nrm[:], lnq[:], AF.Exp, scale=0.5)
                den = wrk.tile(ROW, f32, tag="den")
                if rprev is None:
                    nc.vector.tensor_scalar_add(den[:], qp[:], 1.0)
                    num = nrm
                else:
                    r2 = wrk.tile(ROW, f32, tag="r2")
                    nc.vector.tensor_tensor(r2[:], rprev[:], rprev[:], OP.mult)
                    qq = wrk.tile(ROW, f32, tag="qq")
                    nc.vector.tensor_tensor(qq[:], qp[:], r2[:], OP.mult)
                    nc.vector.tensor_scalar_add(den[:], qq[:], 1.0)
                    num = wrk.tile(ROW, f32, tag="num")
                    nc.vector.tensor_tensor(num[:], nrm[:], r2[:], OP.mult)
                rden = wrk.tile(ROW, f32, tag="rden")
                nc.vector.reciprocal(rden[:], den[:])
                coef = wrk.tile(ROW, f32, tag="coef")
                nc.vector.tensor_tensor(coef[:], num[:], rden[:], OP.mult)
                return coef

            rprev = None      # 1/sum(exp(logits)) of previous iteration
            lg = None         # logits (n-part, b-free), exact
            ex = None
            for it in range(ITERATIONS):
                # S^T[o,b] (unnormalized for it>0) and u-related products
                if it == 0:
                    ur = wrk.tile([O, B_LOC], f32, tag="ur")
                    nc.vector.tensor_scalar_mul(ur[:], xt[:], wbar[:])
                    th = wrk.tile([O, B_LOC], f32, tag="th")
                    nc.vector.tensor_scalar_mul(th[:], x2t[:], wbar[:])
                else:
                    ps_s = ps.tile([O, B_LOC], f32, tag="ps_a", name="ps_s")
                    nc.tensor.matmul(ps_s[:], w_no[:], ex[:],
                                     start=True, stop=True)
                    ur = wrk.tile([O, B_LOC], f32, tag="ur")
                    nc.vector.tensor_tensor(ur[:], xt[:], ps_s[:], OP.mult)
                    if it < ITERATIONS - 1:
                        th = wrk.tile([O, B_LOC], f32, tag="th")
                        nc.vector.tensor_tensor(th[:], x2t[:], ps_s[:], OP.mult)
                # q_raw[b] = sum_o ur^2  (true q = q_raw * rprev^2)
                sq = wrk.tile([O, B_LOC], f32, tag="sq")
                nc.scalar.activation(sq[:], ur[:], AF.Square)
                ps_q = ps.tile(ROW, f32, tag="ps_q")
                nc.tensor.matmul(ps_q[:], ones_c[:], sq[:],
                                 start=True, stop=True)
                coef = rowchain(ps_q, rprev)
                if debug and it > 0:
                    nc.sync.dma_start(dbg_r[f"cf{it}"][:], coef[:])

                if it < ITERATIONS - 1:
                    # delta^T[n,b] = matmul(W^T, X2T*S) ; logits update is
                    # delta * coef broadcast over n partitions (rowchain's
                    # coef already carries the rprev normalization factor)
                    ps_d = ps.tile([N_CAPS, B_LOC], f32, tag="ps_d")
                    nc.tensor.matmul(ps_d[:], wt_on[:], th[:],
                                     start=True, stop=True)
                    ds = wrk.tile([N_CAPS, B_LOC], f32, tag="ds")
                    nc.scalar.activation(ds[:], ps_d[:], AF.Copy)
                    ps_b = ps.tile([N_CAPS, B_LOC], f32, tag="ps_d",
                                   name="ps_b")
                    nc.tensor.matmul(ps_b[:], ones_r[:, :N_CAPS], coef[:],
                                     start=True, stop=True)
                    dd = wrk.tile([N_CAPS, B_LOC], f32, tag=f"dd{it}")
                    nc.vector.tensor_tensor(dd[:], ds[:], ps_b[:], OP.mult)
                    if lg is None:
                        lg2 = dd
                    else:
                        lg2 = wrk.tile([N_CAPS, B_LOC], f32, tag="lg2")
                        nc.vector.tensor_tensor(lg2[:], lg[:], dd[:], OP.add)
                    lg = lg2
                    if debug and it == 0:
                        nc.sync.dma_start(dbg_lg[:], lg[:])
                    # bias shifts the softmax (invariant) to keep the
                    # deferred-normalization q_raw inside the ACT Ln HW
                    # range of +-2^64: logits reach ~35, so exp must be
                    # shifted enough that q_raw ~ e^(2*(35-21))*O(100)
                    # stays under 2^64 (at -12 one batch row's norm came
                    # back as garbage on HW).
                    ex = wrk.tile([N_CAPS, B_LOC], f32, tag="ex")
                    nc.scalar.activation(ex[:], lg[:], AF.Exp, bias=nbias[:])
                    ps_m = ps.tile(ROW, f32, tag="ps_q", name="ps_m")
                    nc.tensor.matmul(ps_m[:], ones_c[:N_CAPS, :], ex[:],
                                     start=True, stop=True)
                    rnew = wrk.tile(ROW, f32, tag="rnew")
                    nc.vector.reciprocal(rnew[:], ps_m[:])
                    rprev = rnew
                    if debug:
                        if it == 0:
                            nc.sync.dma_start(dbg_ex1[:], ex[:])
                        else:
                            nc.sync.dma_start(dbg_lg2[:], lg[:])
                        nc.sync.dma_start(dbg_r[f"r{it+1}"][:], rnew[:])
                else:
                    # out[b,o] = coef[b] * ur[o,b], transposed (coef
                    # already carries the rprev normalization factor)
                    ps_f = ps.tile([O, B_LOC], f32, tag="ps_a", name="ps_f")
                    nc.tensor.matmul(ps_f[:], ones_r[:], coef[:],
                                     start=True, stop=True)
                    outt = wrk.tile([O, B_LOC], f32, tag="outt")
                    nc.vector.tensor_tensor(outt[:], ur[:], ps_f[:], OP.mult)
                    ps_o = ps.tile([B_LOC, O], f32, tag="ps_o")
                    nc.tensor.transpose(ps_o[:], outt[:], ident[:O, :O])
                    out_sb = wrk.tile([B_LOC, O], f32, tag="out_sb")
                    nc.vector.tensor_copy(out_sb[:], ps_o[:])
                    nc.sync.dma_start(out_d[:], out_sb[:])

    nc.compile()
    return nc


def run_with_results(x: np.ndarray, caps_weights: np.ndarray, **run_kwargs):
    """Run the SPMD kernel; returns (output (256,1,128), BassKernelResults)."""
    from concourse.bass_utils import run_bass_kernel_spmd

    if "nc" not in _cache:
        _cache["nc"] = _build()
    nc = _cache["nc"]

    x = np.ascontiguousarray(x, dtype=np.float32)
    caps_weights = np.ascontiguousarray(caps_weights, dtype=np.float32)
    cst = np.zeros((128, 2 * B_LOC - 1), dtype=np.float32)
    cst[:, B_LOC - 1] = 1.0

    in_maps = []
    for c in range(N_CORES):
        in_maps.append({
            "x": np.ascontiguousarray(x[:, c * B_LOC:(c + 1) * B_LOC, :]),
            "caps_weights": caps_weights,
            "cst": cst,
        })
    res = run_bass_kernel_spmd(nc, in_maps, core_ids=list(range(N_CORES)),
                               **run_kwargs)
    out = np.concatenate([res.results[c]["out"] for c in range(N_CORES)], axis=0)
    return out.reshape(BATCH, 1, O), res


def kernel(x: np.ndarray, caps_weights: np.ndarray) -> np.ndarray:
    out, _ = run_with_results(x, caps_weights)
    return out


# revision 32
# speedup vs baseline: 1.4233x; 1.4233x over previous
"""Trainium2 Bass kernel for nn_Capsule: capsule routing head.

Math: the einsum 'nco,pbo->bno' factorizes as xp[b,n,o] = W[n,o] * X[b,o]
with W = caps_weights.sum(c) (64x128) and X = x.sum(p) (256x128), so the
kernel is a memory-bound reduction of x (151 MB) followed by a tiny
per-batch routing loop.

Sharding: data-parallel over batch (dim 1 of x), 32 batch elements per
core; caps_weights replicated; no cross-core communication.

Per-core pipeline (v2):
  - x is streamed as 9 slabs of 128 partition-rows.  128 is the only
    partition count that gets the native per-port SDMA engine split
    (~24.5 GB/s/engine measured); other counts fall into contiguous
    block splits that convoy on SBUF ports (120 -> 15 engines at ~15
    GB/s, 124 -> only 4 engines).
  - Slabs 0-5 stream full-width (16KB/partition descriptors, line
    rate); the tail block {s6, s7, s8, rem} is split into 4 batch
    ranges of 8 (4KB descriptors -- 2KB ones measured only ~66% of
    line rate), so only the last range's reduction matmuls are exposed
    at stream end.  cst/caps_weights/remainder chunks ride the gpsimd
    (SWDGE) queue so the HWDGE rings carry nothing but x slabs.
  - Reduction via fp32r matmuls with one-hot-column stationary matrices
    into a single psum bank (32, 512), in three phases matched to
    stream arrival (slabs 0-2, 3-5, then 6-9 per range).
  - Routing runs in o-on-partitions layout: partition reductions and
    per-batch broadcasts are tiny matmuls (ones stationaries), softmax
    normalization is deferred into per-batch row scalars, and no
    mid-routing transposes are needed.  sqrt(q) = Exp(0.5*Ln(q)) keeps
    every ACT op in one table ('natural_log_exp_and_others').
"""

import numpy as np

# ---- problem constants (hardcoded per contract) ----
P_TOT = 1152
BATCH = 256
O = 128
N_CAPS = 64
CAPS_DIM = 16
ITERATIONS = 3
N_CORES = 8
B_LOC = BATCH // N_CORES          # 32 batch elements per core

PR = 128                          # partition rows per slab (native DMA split)
NS = 9                            # slabs (1152 = 9*128, no remainder)
NFULL = 6                         # slabs streamed full-width (16KB descriptors)
NR = 4                            # batch ranges for the tail block {s6..s8}
BB = B_LOC // NR                  # batch elements per range (8)
SW = B_LOC * O                    # free width per slab position (4096)

_cache = {}


def _pin_act_table():
    """Force every ACT function onto the one table containing
    Exp+Ln+Square+Copy, so the kernel needs a single ACT_TABLE_LOAD."""
    import functools
    import concourse.hw_specs as hw_specs
    import concourse.bacc as bacc_mod

    if getattr(hw_specs.get_activation_tables, "_capsule_pinned", False):
        return
    orig = hw_specs.get_activation_tables

    @functools.cache
    def pinned(module_arch):
        tabs = orig(module_arch)
        keep = None
        for name, fns in tabs.items():
            names = {f.name for f in fns}
            if {"Exp", "Ln", "Square", "Copy", "Identity"} <= names:
                keep = name
                break
        if keep is None:
            return tabs
        return {n: (fns if n == keep else type(fns)()) for n, fns in tabs.items()}

    pinned._capsule_pinned = True
    hw_specs.get_activation_tables = pinned
    bacc_mod.get_activation_tables = pinned


def _build(debug=False):
    _pin_act_table()
    import concourse.bacc as bacc
    import concourse.tile as tile
    import concourse.mybir as mybir
    from concourse.masks import make_identity

    f32 = mybir.dt.float32
    f32r = mybir.dt.float32r
    AX = mybir.AxisListType
    AF = mybir.ActivationFunctionType
    OP = mybir.AluOpType

    nc = bacc.Bacc(None, target_bir_lowering=False)

    # x declared f32r: same bytes as fp32, feeds the fast fp32r matmul
    # path with no cast.
    x_in = nc.dram_tensor("x", [P_TOT, B_LOC, O], f32r, kind="ExternalInput")
    w_in = nc.dram_tensor("caps_weights", [N_CAPS, CAPS_DIM, O], f32,
                          kind="ExternalInput")
    # one-hot stationary source: (128, 63) with ones in column 31, so
    # cst[:p, 31-b : 63-b] is the one-hot-column-b matrix on p rows.
    cst_in = nc.dram_tensor("cst", [128, 2 * B_LOC - 1], f32r,
                            kind="ExternalInput")
    out_d = nc.dram_tensor("out", [B_LOC, O], f32, kind="ExternalOutput")
    if debug:
        dbg_x32 = nc.dram_tensor("dbg_x32", [B_LOC, O], f32,
                                 kind="ExternalOutput")
        dbg_lg = nc.dram_tensor("dbg_lg", [N_CAPS, B_LOC], f32,
                                kind="ExternalOutput")
        dbg_ex1 = nc.dram_tensor("dbg_ex1", [N_CAPS, B_LOC], f32,
                                 kind="ExternalOutput")
        dbg_r = {}
        for nm in ("r1", "r2", "cf1", "cf2"):
            dbg_r[nm] = nc.dram_tensor(f"dbg_{nm}", [1, B_LOC], f32,
                                       kind="ExternalOutput")
        dbg_lg2 = nc.dram_tensor("dbg_lg2", [N_CAPS, B_LOC], f32,
                                 kind="ExternalOutput")

    with tile.TileContext(nc) as tc:
        with (
            tc.tile_pool(name="xin", bufs=1) as xpool,
            tc.tile_pool(name="wrk", bufs=1) as wrk,
            tc.tile_pool(name="small", bufs=1) as small,
            tc.tile_pool(name="ps", bufs=1, space="PSUM") as ps,
        ):
            # ---- constants / weights ride the SWDGE (gpsimd) queue so
            # the two HWDGE rings carry nothing but x chunks.
            zpat = small.tile([128, 2 * B_LOC - 1], f32r)
            nc.gpsimd.dma_start(zpat[:], cst_in[:])
            w_sb = wrk.tile([N_CAPS, CAPS_DIM * O], f32)
            nc.gpsimd.dma_start(w_sb[:], w_in.rearrange("n c o -> n (c o)"))

            # ---- x stream.  Slab layout is b-major: position s holds
            # (120, 32b, 128o) at cols [s*4096, (s+1)*4096).  Slabs 0-5
            # stream full-width (16KB/partition descriptors, line rate);
            # the tail block {s6, s7, s8, rem} is split into 4 batch
            # ranges of 8 (4KB descriptors) so only the last range's
            # reduction matmuls are exposed at stream end.  The 72-row
            # remainder lives zero-padded at position 9; its chunks ride
            # the SWDGE queue.
            xa = xpool.tile([PR, NS * SW], f32r, name="xa")
            xa_sl = xa[:].rearrange("p (s b o) -> p s b o", s=NS, b=B_LOC)
            engs = [nc.sync, nc.scalar]
            for s in range(NFULL):
                engs[s % 2].dma_start(
                    xa_sl[:, s, :, :],
                    x_in[s * PR:(s + 1) * PR, :, :])
            for r in range(NR):
                for s in range(NFULL, NS):
                    engs[(s + r) % 2].dma_start(
                        xa_sl[:, s, r * BB:(r + 1) * BB, :],
                        x_in[s * PR:(s + 1) * PR, r * BB:(r + 1) * BB, :])

            # ---- small constants ----
            ones_c = small.tile([128, 1], f32)      # column of ones
            nc.vector.memset(ones_c[:], 1.0)
            ones_r = small.tile([1, 128], f32)      # row of ones
            nc.vector.memset(ones_r[:], 1.0)
            nbias = small.tile([N_CAPS, 1], f32)    # softmax shift (see Exp)
            nc.vector.memset(nbias[:], -21.0)
            ident = small.tile([128, 128], f32)
            make_identity(nc, ident[:])

            # ---- capsule weight prep (overlaps the x stream) ----
            t1 = wrk.tile([N_CAPS, 8 * O], f32)
            nc.vector.tensor_tensor(t1[:], w_sb[:, :8 * O], w_sb[:, 8 * O:], OP.add)
            t2 = wrk.tile([N_CAPS, 4 * O], f32)
            nc.vector.tensor_tensor(t2[:], t1[:, :4 * O], t1[:, 4 * O:], OP.add)
            t3 = wrk.tile([N_CAPS, 2 * O], f32)
            nc.vector.tensor_tensor(t3[:], t2[:, :2 * O], t2[:, 2 * O:], OP.add)
            w_no = wrk.tile([N_CAPS, O], f32)          # W[n,o]
            nc.vector.tensor_tensor(w_no[:], t3[:, :O], t3[:, O:], OP.add)

            ps_wt = ps.tile([O, N_CAPS], f32, tag="ps_wt")
            nc.tensor.transpose(ps_wt[:], w_no[:], ident[:N_CAPS, :N_CAPS])
            wt_on = wrk.tile([O, N_CAPS], f32)          # W^T[o,n]
            nc.vector.tensor_copy(wt_on[:], ps_wt[:])
            # wbar[o] = mean_n W[n,o]  (column of per-o means)
            ps_wb = ps.tile([O, 1], f32, tag="ps_wb")
            nc.tensor.matmul(ps_wb[:], w_no[:], ones_c[:N_CAPS, :],
                             start=True, stop=True)
            wbar = wrk.tile([O, 1], f32)
            nc.scalar.activation(wbar[:], ps_wb[:], AF.Copy, scale=1.0 / N_CAPS)

            # ---- reduction: X[b,o] = sum_p x[p,b,o] ----
            # one-hot-column stationaries land row b of psum.  Three
            # phases matched to stream arrival: P1 = slabs 0-2 (384-wide,
            # after slab 2 lands), P2 = slabs 3-5, P3 per range = slabs
            # 6-8.  psum sub-column j*128+o accumulates slabs with
            # s = j mod 3; all 96 matmuls accumulate into ONE bank.
            ps_x = ps.tile([B_LOC, 3 * O], f32, tag="ps_x")
            first = True
            for b in range(B_LOC):
                st = zpat[0:PR, B_LOC - 1 - b: 2 * B_LOC - 1 - b]
                nc.tensor.matmul(ps_x[:], st, xa_sl[:, 0:3, b, :],
                                 start=first, stop=False,
                                 skip_group_check=True)
                first = False
            for b in range(B_LOC):
                st = zpat[0:PR, B_LOC - 1 - b: 2 * B_LOC - 1 - b]
                nc.tensor.matmul(ps_x[:], st, xa_sl[:, 3:6, b, :],
                                 start=False, stop=False,
                                 skip_group_check=True)
            for r in range(NR):
                for bb in range(BB):
                    b = r * BB + bb
                    st = zpat[0:PR, B_LOC - 1 - b: 2 * B_LOC - 1 - b]
                    last = (r == NR - 1 and bb == BB - 1)
                    nc.tensor.matmul(ps_x[:], st, xa_sl[:, 6:9, b, :],
                                     start=False, stop=last,
                                     skip_group_check=True)

            # ---- X prep: X (b-part) -> XT, X2T (o-part) ----
            x32 = wrk.tile([B_LOC, O], f32)
            nc.vector.tensor_reduce(
                x32[:], ps_x[:].rearrange("p (j o) -> p o j", j=3),
                AX.X, OP.add)
            if debug:
                nc.sync.dma_start(dbg_x32[:], x32[:])
            ps_a = ps.tile([O, B_LOC], f32, tag="ps_a")
            nc.tensor.transpose(ps_a[:], x32[:], ident[:B_LOC, :B_LOC])
            xt = wrk.tile([O, B_LOC], f32)              # X^T[o,b]
            nc.vector.tensor_copy(xt[:], ps_a[:])
            x2t = wrk.tile([O, B_LOC], f32)             # (X^T)^2
            nc.scalar.activation(x2t[:], xt[:], AF.Square)

            # ---- routing, o-on-partitions ----
            # row scalars live on (1, B_LOC); partition reductions and
            # broadcasts are matmuls with ones stationaries.
            ROW = [1, B_LOC]

            def rowchain(qp, rprev):
                """scale-row coef = sqrt(q)/(1+q) (x rprev^2 deferred
                softmax normalization; rprev=None for iteration 0).
                qp: (1,32) psum with raw |u|^2 (missing rprev^2)."""
                lnq = wrk.tile(ROW, f32, tag="lnq")
                nc.scalar.activation(lnq[:], qp[:], AF.Ln)
                nrm = wrk.tile(ROW, f32, tag="nrm")     # sqrt(q_raw)
                nc.scalar.activation(nrm[:], lnq[:], AF.Exp, scale=0.5)
                den = wrk.tile(ROW, f32, tag="den")
                if rprev is None:
                    nc.vector.tensor_scalar_add(den[:], qp[:], 1.0)
                    num = nrm
                else:
                    r2 = wrk.tile(ROW, f32, tag="r2")
                    nc.vector.tensor_tensor(r2[:], rprev[:], rprev[:], OP.mult)
                    qq = wrk.tile(ROW, f32, tag="qq")
                    nc.vector.tensor_tensor(qq[:], qp[:], r2[:], OP.mult)
                    nc.vector.tensor_scalar_add(den[:], qq[:], 1.0)
                    num = wrk.tile(ROW, f32, tag="num")
                    nc.vector.tensor_tensor(num[:], nrm[:], r2[:], OP.mult)
                rden = wrk.tile(ROW, f32, tag="rden")
                nc.vector.reciprocal(rden[:], den[:])
                coef = wrk.tile(ROW, f32, tag="coef")
                nc.vector.tensor_tensor(coef[:], num[:], rden[:], OP.mult)
                return coef

            rprev = None      # 1/sum(exp(logits)) of previous iteration
            lg = None         # logits (n-part, b-free), exact
            ex = None
            for it in range(ITERATIONS):
                # S^T[o,b] (unnormalized for it>0) and u-related products
                if it == 0:
                    ur = wrk.tile([O, B_LOC], f32, tag="ur")
                    nc.vector.tensor_scalar_mul(ur[:], xt[:], wbar[:])
                    th = wrk.tile([O, B_LOC], f32, tag="th")
                    nc.vector.tensor_scalar_mul(th[:], x2t[:], wbar[:])
                else:
                    ps_s = ps.tile([O, B_LOC], f32, tag="ps_a", name="ps_s")
                    nc.tensor.matmul(ps_s[:], w_no[:], ex[:],
                                     start=True, stop=True)
                    ur = wrk.tile([O, B_LOC], f32, tag="ur")
                    nc.vector.tensor_tensor(ur[:], xt[:], ps_s[:], OP.mult)
                    if it < ITERATIONS - 1:
                        th = wrk.tile([O, B_LOC], f32, tag="th")
                        nc.vector.tensor_tensor(th[:], x2t[:], ps_s[:], OP.mult)
                # q_raw[b] = sum_o ur^2  (true q = q_raw * rprev^2)
                sq = wrk.tile([O, B_LOC], f32, tag="sq")
                nc.scalar.activation(sq[:], ur[:], AF.Square)
                ps_q = ps.tile(ROW, f32, tag="ps_q")
                nc.tensor.matmul(ps_q[:], ones_c[:], sq[:],
                                 start=True, stop=True)
                coef = rowchain(ps_q, rprev)
                if debug and it > 0:
                    nc.sync.dma_start(dbg_r[f"cf{it}"][:], coef[:])

                if it < ITERATIONS - 1:
                    # delta^T[n,b] = matmul(W^T, X2T*S) ; logits update is
                    # delta * coef broadcast over n partitions (rowchain's
                    # coef already carries the rprev normalization factor)
                    ps_d = ps.tile([N_CAPS, B_LOC], f32, tag="ps_d")
                    nc.tensor.matmul(ps_d[:], wt_on[:], th[:],
                                     start=True, stop=True)
                    ds = wrk.tile([N_CAPS, B_LOC], f32, tag="ds")
                    nc.scalar.activation(ds[:], ps_d[:], AF.Copy)
                    ps_b = ps.tile([N_CAPS, B_LOC], f32, tag="ps_d",
                                   name="ps_b")
                    nc.tensor.matmul(ps_b[:], ones_r[:, :N_CAPS], coef[:],
                                     start=True, stop=True)
                    dd = wrk.tile([N_CAPS, B_LOC], f32, tag=f"dd{it}")
                    nc.vector.tensor_tensor(dd[:], ds[:], ps_b[:], OP.mult)
                    if lg is None:
                        lg2 = dd
                    else:
                        lg2 = wrk.tile([N_CAPS, B_LOC], f32, tag="lg2")
                        nc.vector.tensor_tensor(lg2[:], lg[:], dd[:], OP.add)
                    lg = lg2
                    if debug and it == 0:
                        nc.sync.dma_start(dbg_lg[:], lg[:])
                    # bias shifts the softmax (invariant) to keep the
                    # deferred-normalization q_raw inside the ACT Ln HW
                    # range of +-2^64: logits reach ~35, so exp must be
                    # shifted enough that q_raw ~ e^(2*(35-21))*O(100)
                    # stays under 2^64 (at -12 one batch row's norm came
                    # back as garbage on HW).
                    ex = wrk.tile([N_CAPS, B_LOC], f32, tag="ex")
                    nc.scalar.activation(ex[:], lg[:], AF.Exp, bias=nbias[:])
                    ps_m = ps.tile(ROW, f32, tag="ps_q", name="ps_m")
                    nc.tensor.matmul(ps_m[:], ones_c[:N_CAPS, :], ex[:],
                                     start=True, stop=True)
                    rnew = wrk.tile(ROW, f32, tag="rnew")
                    nc.vector.reciprocal(rnew[:], ps_m[:])
                    rprev = rnew
                    if debug:
                        if it == 0:
                            nc.sync.dma_start(dbg_ex1[:], ex[:])
                        else:
                            nc.sync.dma_start(dbg_lg2[:], lg[:])
                        nc.sync.dma_start(dbg_r[f"r{it+1}"][:], rnew[:])
                else:
                    # out[b,o] = coef[b] * ur[o,b], transposed (coef
                    # already carries the rprev normalization factor)
                    ps_f = ps.tile([O, B_LOC], f32, tag="ps_a", name="ps_f")
                    nc.tensor.matmul(ps_f[:], ones_r[:], coef[:],
                                     start=True, stop=True)
                    outt = wrk.tile([O, B_LOC], f32, tag="outt")
                    nc.vector.tensor_tensor(outt[:], ur[:], ps_f[:], OP.mult)
                    ps_o = ps.tile([B_LOC, O], f32, tag="ps_o")
                    nc.tensor.transpose(ps_o[:], outt[:], ident[:O, :O])
                    out_sb = wrk.tile([B_LOC, O], f32, tag="out_sb")
                    nc.vector.tensor_copy(out_sb[:], ps_o[:])
                    nc.sync.dma_start(out_d[:], out_sb[:])

    nc.compile()
    return nc


def run_with_results(x: np.ndarray, caps_weights: np.ndarray, **run_kwargs):
    """Run the SPMD kernel; returns (output (256,1,128), BassKernelResults)."""
    from concourse.bass_utils import run_bass_kernel_spmd

    if "nc" not in _cache:
        _cache["nc"] = _build()
    nc = _cache["nc"]

    x = np.ascontiguousarray(x, dtype=np.float32)
    caps_weights = np.ascontiguousarray(caps_weights, dtype=np.float32)
    cst = np.zeros((128, 2 * B_LOC - 1), dtype=np.float32)
    cst[:, B_LOC - 1] = 1.0

    in_maps = []
    for c in range(N_CORES):
        in_maps.append({
            "x": np.ascontiguousarray(x[:, c * B_LOC:(c + 1) * B_LOC, :]),
            "caps_weights": caps_weights,
            "cst": cst,
        })
    res = run_bass_kernel_spmd(nc, in_maps, core_ids=list(range(N_CORES)),
                               **run_kwargs)
    out = np.concatenate([res.results[c]["out"] for c in range(N_CORES)], axis=0)
    return out.reshape(BATCH, 1, O), res


def kernel(x: np.ndarray, caps_weights: np.ndarray) -> np.ndarray:
    out, _ = run_with_results(x, caps_weights)
    return out
